# revision 15
# baseline (speedup 1.0000x reference)
"""Trainium2 Bass kernel for nn_DecoderWithAttention — v2.

2-layer GRU decoder with Bahdanau attention, 12 sequential timesteps.
Data-parallel over batch (64 -> 8 cores x 8), weights replicated.

v2 design (vs baseline): the whole dataflow is transposed so matmul
outputs are [unit-on-partitions, batch-free] — each gate matmul streams
only 8 columns instead of 512, cutting PE streaming rows per step from
~35k to ~5k.  The hidden state lives only in [128, chunk, batch] bf16
layout (no per-step transposes).  Sigmoid is eliminated via
sigma(x) = (1+tanh(x/2))/2 with the 1/2 folded into the r/z weights, so
the Act engine needs only {tanh, exp, copy} — one activation table, no
ACT_TABLE_LOAD switches.  GRU elementwise math uses fused
scalar_tensor_tensor ops.  The attention context ws is computed
explicitly (32 tiny [128,1] matmuls against an e-partitioned copy of
enc) instead of the baseline's encW fold.
"""
import sys
sys.path.insert(0, '/opt/trn_rl_repo')
import numpy as np

B, DEC, F = 64, 12, 32
L, H = 2, 512
E, T = 96, 4
N_CORES = 8
BS = B // N_CORES  # 8 batches per core

_COMPILED = {}


def _f32(x):
    return np.ascontiguousarray(x, dtype=np.float32)


def _bf16(x):
    import ml_dtypes
    return np.ascontiguousarray(np.asarray(x, dtype=np.float32).astype(ml_dtypes.bfloat16))


def _f8(x):
    import ml_dtypes
    return np.ascontiguousarray(np.asarray(x, dtype=np.float32)
                                .astype(ml_dtypes.float8_e4m3))


def build_nc():
    import concourse.bass as bass
    import concourse.tile as tile
    from concourse import mybir
    from concourse.vector_clock import ScopedClock

    f32 = mybir.dt.float32
    bf16 = mybir.dt.bfloat16
    f8 = mybir.dt.float8e4
    AF = mybir.ActivationFunctionType
    OP = mybir.AluOpType

    # --- patch: the TileContext exit drain gets >1 sem wait, which this
    # walrus rejects ("Too many sync wait commands"); split into
    # single-wait drains. ---
    def patched_drain(self, tick_clock, wait_clock):
        nc = self.nc
        drain_inst = nc.sync.drain()
        wait_clock.add_sem_waits(
            drain_inst.ins, ScopedClock({None: tick_clock.global_clock}))
        si = drain_inst.ins.sync_info
        waits = list(si.on_wait or [])
        if len(waits) > 1:
            SyncInfo = type(si)
            drain_inst.ins.sync_info = SyncInfo(
                on_wait=[waits[0]], on_update=list(si.on_update or []))
            for w in waits[1:]:
                d2 = nc.sync.drain()
                d2.ins.sync_info = SyncInfo(on_wait=[w], on_update=[])
        nc.all_engine_barrier()
        assert self.sems is not None
        popped = nc._tile_sem_poison_stack.pop()
        assert popped is self._sem_poison
        nc.clear_and_free_semaphores(list(self.sems.allocated().values()))
        nc.all_engine_barrier()

    tile.TileContext._drain_and_barrier = patched_drain

    nc = bass.Bass()

    def P(name, shape, dt=bf16):
        return nc.declare_dram_parameter(name, list(shape), dt, isOutput=False)

    # inputs/weights, in DMA priority order.  aw/al0/al1 are fp8 at 64x
    # scale; the 1/64 is recovered for free via the Act `scale` input at
    # each tanh (the pre-tanh ops are all linear).
    encT_e = P("encT", [128, 4, BS * E])      # enc, h-partitioned
    wae_e = P("wae", [128, 4, H], f8)         # waeT x64 (encP, needed first)
    wah_e = P("wah", [128, 4, H], f8)         # wahT x64 (q)
    h1T_e = P("h1T", [128, 4, BS])
    battn_e = P("battn", [1, H])              # x64
    ones8_e = P("ones8", [1, BS])
    ones96_e = P("ones96", [E, 128])
    v_e = P("v", [128, 4, 4, BS])             # block-diag v for dense scores
    h0T_e = P("h0T", [128, 4, BS])
    cb0_e = P("cb0", [128, BS])               # [cur(0:32) | one(32) | 0...]
    inT_e = P("inT", [32, DEC, BS])
    wo_e = P("wo", [128, 9, T])               # Wout chunks [h1(0:4)|ws(4:8)|cb]
    al0_e = P("al0", [128, 28, H], f8)        # l0 r(0:9) z(9:18) ni(18:23) nh(23:28), x64 (nh x32)
    encE_e = P("encE", [E, BS, 4, 128])       # enc, e-partitioned
    al1_e = P("al1", [128, 28, H], f8)        # l1 same chunk order
    out_e = nc.declare_dram_parameter("out", [T, DEC, BS], f32, isOutput=True)

    with tile.TileContext(nc) as tc:
        with tc.tile_pool(name="wts", bufs=1) as wts, \
             tc.tile_pool(name="st", bufs=2) as st, \
             tc.tile_pool(name="wk", bufs=2) as wk, \
             tc.tile_pool(name="psg", bufs=4, space="PSUM") as psg, \
             tc.tile_pool(name="pss", bufs=1, space="PSUM") as pss:

            def load(ext, shape, dt=bf16):
                t = wts.tile(list(shape), dt, tag=ext.name)
                nc.sync.dma_start(t[:], ext[:])
                return t

            encT = load(encT_e, [128, 4, BS * E])
            wae = load(wae_e, [128, 4, H], f8)
            wah = load(wah_e, [128, 4, H], f8)
            # persistent state: h0T/h1T rotate (bufs=2); cb in-place
            h0T = st.tile([128, 4, BS], bf16, tag="h0T")
            h1T = st.tile([128, 4, BS], bf16, tag="h1T")
            nc.sync.dma_start(h1T[:], h1T_e[:])
            battn = load(battn_e, [1, H])
            ones8 = load(ones8_e, [1, BS])
            ones96 = load(ones96_e, [E, 128])
            v_sb = load(v_e, [128, 4, 4, BS])
            nc.sync.dma_start(h0T[:], h0T_e[:])
            cb = wts.tile([128, BS], bf16, tag="cb")
            nc.sync.dma_start(cb[:], cb0_e[:])
            inT = load(inT_e, [32, DEC, BS])
            wo = load(wo_e, [128, 9, T])
            al0 = load(al0_e, [128, 28, H], f8)
            encE = load(encE_e, [E, BS, 4, 128])
            al1 = load(al1_e, [128, 28, H], f8)
            encP = wts.tile([128, 4, BS * E], bf16, tag="encP")
            outT = wts.tile([T, DEC, BS], f32, tag="outT")

            def emit_q(h1T_cur):
                """q = Wa_h @ h1 + b_attn -> psum pq; then q_sb copy."""
                patt = pss.tile([128, 8, BS], f32, tag="att")
                pq = patt[:, 0:4, :]
                for m in range(4):
                    for k in range(4):
                        nc.tensor.matmul(
                            pq[:, m, :], wah[:, k, m * 128:(m + 1) * 128],
                            h1T_cur[:, k, :], start=(k == 0), stop=False)
                    nc.tensor.matmul(
                        pq[:, m, :], battn[0:1, m * 128:(m + 1) * 128],
                        ones8[0:1, :], start=False, stop=True)
                q_sb = wk.tile([128, 4, BS], bf16, tag="q_sb")
                nc.vector.tensor_copy(q_sb[:], pq[:])
                return patt, q_sb

            # t=0 attention query first so it's not stuck behind setup in
            # the PE queue
            patt_c, q_sb_c = emit_q(h1T)

            def emit_encP(m):
                """one encP h-chunk: 2 psum groups + copies (Act/DVE)."""
                for n2 in range(2):
                    ns = slice(n2 * 384, (n2 + 1) * 384)
                    pe = psg.tile([128, 384], f32, tag="g")
                    for k in range(4):
                        nc.tensor.matmul(
                            pe[:], wae[:, k, m * 128:(m + 1) * 128],
                            encT[:, k, ns], start=(k == 0), stop=(k == 3))
                    if n2 == 0:
                        nc.scalar.activation(encP[:, m, ns], pe[:], AF.Copy)
                    else:
                        nc.vector.tensor_copy(encP[:, m, ns], pe[:])


            def gru_math(prz, pn, phn, h_old, lname):
                """transposed-layout GRU cell update; returns new hT (bf16)."""
                t_rz = wk.tile([128, 8, BS], bf16, tag=f"trz{lname}")
                nc.scalar.activation(t_rz[:], prz[:], AF.Tanh, scale=1.0 / 128)
                tmp1 = wk.tile([128, 4, BS], f32, tag=f"tm1{lname}")
                nc.vector.scalar_tensor_tensor(
                    tmp1[:], t_rz[:, 0:4, :], 1.0, phn[:], OP.add, OP.mult)
                a2 = wk.tile([128, 4, BS], f32, tag=f"a2{lname}")
                nc.vector.tensor_add(a2[:], tmp1[:], pn[:])
                n_t = wk.tile([128, 4, BS], bf16, tag=f"n{lname}")
                nc.scalar.activation(n_t[:], a2[:], AF.Tanh, scale=1.0 / 64)
                v_t = wk.tile([128, 4, BS], f32, tag=f"v{lname}")
                nc.vector.tensor_sub(v_t[:], h_old[:], n_t[:])
                tmp2 = wk.tile([128, 4, BS], f32, tag=f"tm2{lname}")
                nc.vector.scalar_tensor_tensor(
                    tmp2[:], t_rz[:, 4:8, :], 1.0, v_t[:], OP.add, OP.mult)
                h_new = st.tile([128, 4, BS], bf16, tag=f"h{lname}T")
                nc.vector.scalar_tensor_tensor(
                    h_new[:], tmp2[:], 0.5, n_t[:], OP.mult, OP.add)
                return h_new

            # ================= time loop =================
            for t in range(DEC):
                patt, q_sb = patt_c, q_sb_c

                # --- en = tanh((encP64 + q64)/64), chunked; m3 add on
                # Pool, m0-2 on DVE; tanh order = readiness order ---
                ea = wk.tile([128, 4, BS * E], bf16, tag="ea")
                en = wk.tile([128, 4, BS * E], bf16, tag="en")
                pso = pss.tile([128, 48], f32, tag="pso")
                scT = pso[0:E, 0:BS]
                def emit_add(m):
                    nc.vector.tensor_add(
                        ea[:, m, :].rearrange("p (e b) -> p e b", e=E),
                        encP[:, m, :].rearrange("p (e b) -> p e b", e=E),
                        q_sb[:, m, :].unsqueeze(1).broadcast_to((128, E, BS)))

                morder = (0, 1, 2, 3)
                if t == 0:
                    # interleave the one-time encP build with t0's chunks
                    for m in morder:
                        emit_encP(m)
                        emit_add(m)
                else:
                    for m in (0, 1, 2):
                        emit_add(m)
                    emit_add(3)
                for hh in range(2):
                    ms = slice(2 * hh, 2 * hh + 2)
                    nc.scalar.activation(en[:, ms, :], ea[:, ms, :],
                                         AF.Tanh, scale=1.0 / 64)
                enb = en[:].rearrange("p m (e b) -> p m b e", b=BS)
                for b in range(BS):
                    for ki in range(4):
                        nc.tensor.matmul(
                            scT[:, b:b + 1], enb[:, ki, b, :],
                            v_sb[:, ki, 0, 0:1],
                            start=(ki == 0), stop=(ki == 3))

                # --- softmax over E, transposed: exp [96, 8]; z via
                # all-ones matmul onto all 128 partitions; 1/z folded into
                # the wsT copy ---
                expw = wk.tile([E, BS], bf16, tag="expw")
                nc.scalar.activation(expw[:], scT[:], AF.Exp)
                pzs = pso[:, 32:40]
                nc.tensor.matmul(pzs[:], ones96[:], expw[:],
                                 start=True, stop=True)
                zrP = wk.tile([128, BS], f32, tag="zrP")
                nc.vector.reciprocal(zrP[:], pzs[:])

                # --- wsT[:, m, b] = (enc[b].T-chunk @ expw_b) / z_b ---
                pws = patt[:, 4:8, :]
                for b in range(BS):
                    for m in range(4):
                        nc.tensor.matmul(pws[:, m, b:b + 1],
                                         encE[:, b, m, :],
                                         expw[:, b:b + 1],
                                         start=True, stop=True)
                wsT = wk.tile([128, 4, BS], bf16, tag="wsT")
                nc.vector.tensor_mul(
                    wsT[:], pws[:],
                    zrP[:].unsqueeze(1).broadcast_to((128, 4, BS)))

                # --- GRU layer 0 ---
                rhs0 = [h0T[:, k, :] for k in range(4)] + \
                       [wsT[:, k, :] for k in range(4)] + [cb[:]]
                pg0 = pss.tile([128, 16, BS], f32, tag="g0")
                prz0 = pg0[:, 0:8, :]
                pn0 = pg0[:, 8:12, :]
                phn0 = pg0[:, 12:16, :]
                # h/cb chunks first so PE starts before wsT lands
                rz_ord = [0, 1, 2, 3, 8, 4, 5, 6, 7]
                for m in range(4):
                    for j, i in enumerate(rz_ord):
                        nc.tensor.matmul(
                            prz0[:, m, :], al0[:, i, m * 128:(m + 1) * 128],
                            rhs0[i], start=(j == 0), stop=(j == 8))
                    for j, i in enumerate(rz_ord):
                        nc.tensor.matmul(
                            prz0[:, 4 + m, :],
                            al0[:, 9 + i, m * 128:(m + 1) * 128],
                            rhs0[i], start=(j == 0), stop=(j == 8))
                    for j, (ci, rh) in enumerate(
                            [(4, rhs0[8]), (0, rhs0[4]), (1, rhs0[5]),
                             (2, rhs0[6]), (3, rhs0[7])]):
                        nc.tensor.matmul(
                            pn0[:, m, :], al0[:, 18 + ci, m * 128:(m + 1) * 128],
                            rh, start=(j == 0), stop=(j == 4))
                    for i, rh in enumerate([rhs0[0], rhs0[1], rhs0[2],
                                            rhs0[3], rhs0[8]]):
                        nc.tensor.matmul(
                            phn0[:, m, :], al0[:, 23 + i, m * 128:(m + 1) * 128],
                            rh, start=(i == 0), stop=(i == 4))
                h0T = gru_math(prz0, pn0, phn0, h0T, "0")

                # --- GRU layer 1 ---
                rhs1 = [h0T[:, k, :] for k in range(4)] + \
                       [h1T[:, k, :] for k in range(4)] + [cb[:]]
                pg1 = pss.tile([128, 16, BS], f32, tag="g1")
                prz1 = pg1[:, 0:8, :]
                pn1 = pg1[:, 8:12, :]
                phn1 = pg1[:, 12:16, :]
                for m in range(4):
                    for i in range(9):
                        nc.tensor.matmul(
                            prz1[:, m, :], al1[:, i, m * 128:(m + 1) * 128],
                            rhs1[i], start=(i == 0), stop=(i == 8))
                    for i in range(9):
                        nc.tensor.matmul(
                            prz1[:, 4 + m, :],
                            al1[:, 9 + i, m * 128:(m + 1) * 128],
                            rhs1[i], start=(i == 0), stop=(i == 8))
                    for i, rh in enumerate([rhs1[0], rhs1[1], rhs1[2],
                                            rhs1[3], rhs1[8]]):
                        nc.tensor.matmul(
                            pn1[:, m, :], al1[:, 18 + i, m * 128:(m + 1) * 128],
                            rh, start=(i == 0), stop=(i == 4))
                    for i, rh in enumerate([rhs1[4], rhs1[5], rhs1[6],
                                            rhs1[7], rhs1[8]]):
                        nc.tensor.matmul(
                            phn1[:, m, :], al1[:, 23 + i, m * 128:(m + 1) * 128],
                            rh, start=(i == 0), stop=(i == 4))
                h1T = gru_math(prz1, pn1, phn1, h1T, "1")

                # --- q for t+1 first (only needs h1T), then out proj ---
                if t < DEC - 1:
                    patt_c, q_sb_c = emit_q(h1T)
                po = pso[0:T, 40:48]  # [4, 8] region
                rhso = [h1T[:, k, :] for k in range(4)] + \
                       [wsT[:, k, :] for k in range(4)] + [cb[:]]
                for i in range(9):
                    nc.tensor.matmul(po[:], wo[:, i, :], rhso[i],
                                     start=(i == 0), stop=(i == 8))
                nc.vector.tensor_copy(outT[:, t, :], po[:])

                # --- cur update for next step ---
                if t < DEC - 1:
                    nc.gpsimd.tensor_copy(cb[0:32, :], inT[:, t, :])
                    nc.vector.tensor_copy(cb[0:T, :], po[:])

            nc.sync.dma_start(out_e[:], outT[:])

    # --- post-pass: cap every instruction at one sync wait by hoisting
    # extras onto same-engine NoOps inserted just before it. ---
    from concourse import mybir
    ctr = 0
    f = nc.m.functions[0]
    for blk in f.blocks:
        il = blk.instructions
        i = 0
        while i < len(il):
            inst = il[i]
            si = inst.sync_info
            waits = list(si.on_wait) if si is not None and si.on_wait else []
            if len(waits) > 1:
                SyncInfo = type(si)
                inst.sync_info = SyncInfo(
                    on_wait=[waits[-1]], on_update=list(si.on_update or []))
                for w in waits[:-1]:
                    nop = mybir.InstNoOp(name=f"I-nopw-{ctr}")
                    ctr += 1
                    nop.engine = inst.engine
                    nop.sync_info = SyncInfo(on_wait=[w], on_update=[])
                    nc.register_instruction(nop)
                    il.insert(i, nop)
                    i += 1
            i += 1

    return nc


def _prep_inputs(inputs, hidden, enc_outputs, target_indices,
                 W_attn, b_attn, v_attn,
                 gru_Wi0, gru_Wh0, gru_bi0, gru_bh0,
                 gru_Wi1, gru_Wh1, gru_bi1, gru_bh1,
                 W_out, b_out):
    """Build per-core input maps (host-side layout prep only)."""
    ti = np.asarray(target_indices)
    assert np.array_equal(ti, np.arange(T)), \
        "kernel specialized for target_indices == arange(T)"

    W_attn = np.asarray(W_attn, np.float32)
    Wa_h, Wa_e = W_attn[:, :H], W_attn[:, H:]
    Wi0 = np.asarray(gru_Wi0, np.float32); Wh0 = np.asarray(gru_Wh0, np.float32)
    bi0 = np.asarray(gru_bi0, np.float32); bh0 = np.asarray(gru_bh0, np.float32)
    Wi1 = np.asarray(gru_Wi1, np.float32); Wh1 = np.asarray(gru_Wh1, np.float32)
    bi1 = np.asarray(gru_bi1, np.float32); bh1 = np.asarray(gru_bh1, np.float32)
    W_out = np.asarray(W_out, np.float32); b_out = np.asarray(b_out, np.float32)

    # attention weights: aw[p, k, m*128+c]: k 0-3 = Wa_h, 4-7 = Wa_e
    wah = np.zeros((128, 4, H), np.float32)
    wae = np.zeros((128, 4, H), np.float32)
    for k in range(4):
        wah[:, k, :] = Wa_h[:, k * 128:(k + 1) * 128].T
        wae[:, k, :] = Wa_e[:, k * 128:(k + 1) * 128].T
    wah = _f8(wah * 64.0)
    wae = _f8(wae * 64.0)

    # block-diag v
    v_rk = np.asarray(v_attn, np.float32).reshape(4, 128).T   # [128, 4]
    v_h = np.zeros((128, 4, 4, BS), np.float32)
    for p in range(4):
        v_h[:, :, p, 2 * p] = v_rk
        v_h[:, :, p, 2 * p + 1] = v_rk
    v_h = _bf16(v_h)

    def gate_chunks(Wh_part, Wi_ws, Wi_cur, bias_row, scale):
        """9 chunks: h(4) + ws(4) + cb; or pass None to skip groups."""
        out = []
        if Wh_part is not None:
            for k in range(4):
                out.append(scale * Wh_part[:, k * 128:(k + 1) * 128].T)
        if Wi_ws is not None:
            for k in range(4):
                out.append(scale * Wi_ws[:, k * 128:(k + 1) * 128].T)
        cbch = np.zeros((128, Wh_part.shape[0] if Wh_part is not None
                         else Wi_ws.shape[0]), np.float32)
        if Wi_cur is not None:
            cbch[0:F, :] = scale * Wi_cur.T
        cbch[F, :] = scale * bias_row
        out.append(cbch)
        return out

    # layer 0: contraction z-order [h0(4) | ws(4) | cb]
    l0 = []
    l0 += gate_chunks(Wh0[0:512], Wi0[0:512, F:], Wi0[0:512, :F],
                      bi0[0:512] + bh0[0:512], 64.0)            # r: 9
    l0 += gate_chunks(Wh0[512:1024], Wi0[512:1024, F:], Wi0[512:1024, :F],
                      bi0[512:1024] + bh0[512:1024], 64.0)      # z: 9
    l0 += gate_chunks(None, Wi0[1024:1536, F:], Wi0[1024:1536, :F],
                      bi0[1024:1536], 64.0)                     # ni: 5
    l0 += gate_chunks(Wh0[1024:1536], None, None,
                      bh0[1024:1536], 32.0)                     # nh: 5
    al0 = _f8(np.stack(l0, axis=1))                             # [128, 28, 512]

    # layer 1: z-order [h0'(4) | h1(4) | cb(bias only)]
    l1 = []
    l1 += gate_chunks(Wi1[0:512], Wh1[0:512], None,
                      bi1[0:512] + bh1[0:512], 64.0)            # r: 9
    l1 += gate_chunks(Wi1[512:1024], Wh1[512:1024], None,
                      bi1[512:1024] + bh1[512:1024], 64.0)      # z: 9
    l1 += gate_chunks(Wi1[1024:1536], None, None, bi1[1024:1536], 64.0)  # ni: 5
    l1 += gate_chunks(Wh1[1024:1536], None, None, bh1[1024:1536], 32.0)  # nh: 5
    al1 = _f8(np.stack(l1, axis=1))

    # out projection chunks [h1(4) | ws(4) | cb]
    woc = []
    for k in range(4):
        woc.append(W_out[:, k * 128:(k + 1) * 128].T)
    for k in range(4):
        woc.append(W_out[:, H + k * 128:H + (k + 1) * 128].T)
    cbo = np.zeros((128, T), np.float32)
    cbo[0:F, :] = W_out[:, 2 * H:2 * H + F].T
    cbo[F, :] = b_out
    woc.append(cbo)
    wo = _bf16(np.stack(woc, axis=1))                           # [128, 9, 4]

    battn = _bf16(b_attn[None, :] * 64.0)
    ones8 = _bf16(np.ones((1, BS), np.float32))
    ones96 = _bf16(np.ones((E, 128), np.float32))

    inputs = np.asarray(inputs, np.float32)
    hidden = np.asarray(hidden, np.float32)
    enc_outputs = np.asarray(enc_outputs, np.float32)

    in_maps = []
    for c in range(N_CORES):
        s = slice(c * BS, (c + 1) * BS)
        encc = enc_outputs[s]                      # [8, 96, 512]
        # encT columns (e, b)-major so the q-broadcast add has b packed
        encT = _bf16(encc.transpose(2, 1, 0).reshape(4, 128, E * BS)
                     .transpose(1, 0, 2))
        # encE[e, b, m, c] = enc[b, e, m*128+c]
        encE = _bf16(encc.reshape(BS, E, 4, 128).transpose(1, 0, 2, 3))
        h0 = hidden[0, s]                          # [8, 512]
        h1 = hidden[1, s]
        cb0 = np.zeros((128, BS), np.float32)
        cb0[0:F, :] = inputs[s, 0, :].T
        cb0[F, :] = 1.0
        in_maps.append({
            "encT": encT, "encE": encE,
            "wae": wae, "wah": wah,
            "battn": battn, "ones8": ones8, "ones96": ones96,
            "v": v_h, "al0": al0, "al1": al1, "wo": wo,
            "h0T": _bf16(h0.T.reshape(4, 128, BS).transpose(1, 0, 2)),
            "h1T": _bf16(h1.T.reshape(4, 128, BS).transpose(1, 0, 2)),
            "cb0": _bf16(cb0),
            "inT": _bf16(inputs[s].transpose(2, 1, 0)),
        })
    return in_maps


def get_nc():
    if "nc" not in _COMPILED:
        _COMPILED["nc"] = build_nc()
    return _COMPILED["nc"]


def kernel(**inputs):
    from concourse.bass_utils import run_bass_kernel_spmd
    nc = get_nc()
    in_maps = _prep_inputs(**inputs)
    res = run_bass_kernel_spmd(nc, in_maps, list(range(N_CORES)))
    # out_e is [T, DEC, BS] per core -> full [B, DEC, T]
    out = np.concatenate(
        [res.results[c]["out"].transpose(2, 1, 0) for c in range(N_CORES)],
        axis=0)
    return np.ascontiguousarray(out, dtype=np.float32)


# revision 17
# speedup vs baseline: 1.0209x; 1.0209x over previous
"""Trainium2 Bass kernel for nn_DecoderWithAttention — v2.

2-layer GRU decoder with Bahdanau attention, 12 sequential timesteps.
Data-parallel over batch (64 -> 8 cores x 8), weights replicated.

v2 design (vs baseline): the whole dataflow is transposed so matmul
outputs are [unit-on-partitions, batch-free] — each gate matmul streams
only 8 columns instead of 512, cutting PE streaming rows per step from
~35k to ~5k.  The hidden state lives only in [128, chunk, batch] bf16
layout (no per-step transposes).  Sigmoid is eliminated via
sigma(x) = (1+tanh(x/2))/2 with the 1/2 folded into the r/z weights, so
the Act engine needs only {tanh, exp, copy} — one activation table, no
ACT_TABLE_LOAD switches.  GRU elementwise math uses fused
scalar_tensor_tensor ops.  The attention context ws is computed
explicitly (32 tiny [128,1] matmuls against an e-partitioned copy of
enc) instead of the baseline's encW fold.
"""
import sys
sys.path.insert(0, '/opt/trn_rl_repo')
import numpy as np

B, DEC, F = 64, 12, 32
L, H = 2, 512
E, T = 96, 4
N_CORES = 8
BS = B // N_CORES  # 8 batches per core

_COMPILED = {}


def _f32(x):
    return np.ascontiguousarray(x, dtype=np.float32)


def _bf16(x):
    import ml_dtypes
    return np.ascontiguousarray(np.asarray(x, dtype=np.float32).astype(ml_dtypes.bfloat16))


def _f8(x):
    import ml_dtypes
    return np.ascontiguousarray(np.asarray(x, dtype=np.float32)
                                .astype(ml_dtypes.float8_e4m3))


def build_nc():
    import concourse.bass as bass
    import concourse.tile as tile
    from concourse import mybir
    from concourse.vector_clock import ScopedClock

    f32 = mybir.dt.float32
    bf16 = mybir.dt.bfloat16
    f8 = mybir.dt.float8e4
    AF = mybir.ActivationFunctionType
    OP = mybir.AluOpType

    # --- patch: the TileContext exit drain gets >1 sem wait, which this
    # walrus rejects ("Too many sync wait commands"); split into
    # single-wait drains. ---
    def patched_drain(self, tick_clock, wait_clock):
        nc = self.nc
        drain_inst = nc.sync.drain()
        wait_clock.add_sem_waits(
            drain_inst.ins, ScopedClock({None: tick_clock.global_clock}))
        si = drain_inst.ins.sync_info
        waits = list(si.on_wait or [])
        if len(waits) > 1:
            SyncInfo = type(si)
            drain_inst.ins.sync_info = SyncInfo(
                on_wait=[waits[0]], on_update=list(si.on_update or []))
            for w in waits[1:]:
                d2 = nc.sync.drain()
                d2.ins.sync_info = SyncInfo(on_wait=[w], on_update=[])
        nc.all_engine_barrier()
        assert self.sems is not None
        popped = nc._tile_sem_poison_stack.pop()
        assert popped is self._sem_poison
        nc.clear_and_free_semaphores(list(self.sems.allocated().values()))
        nc.all_engine_barrier()

    tile.TileContext._drain_and_barrier = patched_drain

    nc = bass.Bass()

    def P(name, shape, dt=bf16):
        return nc.declare_dram_parameter(name, list(shape), dt, isOutput=False)

    # inputs/weights, in DMA priority order.  aw/al0/al1 are fp8 at 64x
    # scale; the 1/64 is recovered for free via the Act `scale` input at
    # each tanh (the pre-tanh ops are all linear).
    encT_e = P("encT", [128, 4, BS * E])      # enc, h-partitioned
    wae_e = P("wae", [128, 4, H], f8)         # waeT x64 (encP, needed first)
    wah_e = P("wah", [128, 4, H], f8)         # wahT x64 (q)
    h1T_e = P("h1T", [128, 4, BS])
    battn_e = P("battn", [1, H])              # x64
    ones8_e = P("ones8", [1, BS])
    ones96_e = P("ones96", [E, 128])
    v_e = P("v", [128, 4, 4, BS])             # block-diag v for dense scores
    h0T_e = P("h0T", [128, 4, BS])
    cb0_e = P("cb0", [128, BS])               # [cur(0:32) | one(32) | 0...]
    inT_e = P("inT", [32, DEC, BS])
    wo_e = P("wo", [128, 9, T])               # Wout chunks [h1(0:4)|ws(4:8)|cb]
    al0_e = P("al0", [128, 28, H], f8)        # l0 r(0:9) z(9:18) ni(18:23) nh(23:28), x64 (nh x32)
    encE_e = P("encE", [E, BS, 4, 128])       # enc, e-partitioned
    al1_e = P("al1", [128, 28, H], f8)        # l1 same chunk order
    out_e = nc.declare_dram_parameter("out", [T, DEC, BS], f32, isOutput=True)

    with tile.TileContext(nc) as tc:
        with tc.tile_pool(name="wts", bufs=1) as wts, \
             tc.tile_pool(name="st", bufs=2) as st, \
             tc.tile_pool(name="wk", bufs=2) as wk, \
             tc.tile_pool(name="psg", bufs=4, space="PSUM") as psg, \
             tc.tile_pool(name="pss", bufs=1, space="PSUM") as pss:

            def load(ext, shape, dt=bf16):
                t = wts.tile(list(shape), dt, tag=ext.name)
                nc.sync.dma_start(t[:], ext[:])
                return t

            encT = load(encT_e, [128, 4, BS * E])
            wae = load(wae_e, [128, 4, H], f8)
            wah = load(wah_e, [128, 4, H], f8)
            # persistent state: h0T/h1T rotate (bufs=2); cb in-place
            h0T = st.tile([128, 4, BS], bf16, tag="h0T")
            h1T = st.tile([128, 4, BS], bf16, tag="h1T")
            nc.sync.dma_start(h1T[:], h1T_e[:])
            battn = load(battn_e, [1, H])
            ones8 = load(ones8_e, [1, BS])
            ones96 = load(ones96_e, [E, 128])
            v_sb = load(v_e, [128, 4, 4, BS])
            nc.sync.dma_start(h0T[:], h0T_e[:])
            cb = wts.tile([128, BS], bf16, tag="cb")
            nc.sync.dma_start(cb[:], cb0_e[:])
            inT = load(inT_e, [32, DEC, BS])
            wo = load(wo_e, [128, 9, T])
            al0 = load(al0_e, [128, 28, H], f8)
            encE = load(encE_e, [E, BS, 4, 128])
            al1 = load(al1_e, [128, 28, H], f8)
            encP = wts.tile([128, 4, BS * E], bf16, tag="encP")
            outT = wts.tile([T, DEC, BS], f32, tag="outT")

            def emit_q(h1T_cur):
                """q = Wa_h @ h1 + b_attn -> psum pq; then q_sb copy."""
                patt = pss.tile([128, 8, BS], f32, tag="att")
                pq = patt[:, 0:4, :]
                for m in range(4):
                    for k in range(4):
                        nc.tensor.matmul(
                            pq[:, m, :], wah[:, k, m * 128:(m + 1) * 128],
                            h1T_cur[:, k, :], start=(k == 0), stop=False)
                    nc.tensor.matmul(
                        pq[:, m, :], battn[0:1, m * 128:(m + 1) * 128],
                        ones8[0:1, :], start=False, stop=True)
                q_sb = wk.tile([128, 4, BS], bf16, tag="q_sb")
                nc.vector.tensor_copy(q_sb[:], pq[:])
                return patt, q_sb

            # t=0 attention query first so it's not stuck behind setup in
            # the PE queue
            patt_c, q_sb_c = emit_q(h1T)

            def emit_encP(m):
                """one encP h-chunk: 2 psum groups + copies (Act/DVE)."""
                for n2 in range(2):
                    ns = slice(n2 * 384, (n2 + 1) * 384)
                    pe = psg.tile([128, 384], f32, tag="g")
                    for k in range(4):
                        nc.tensor.matmul(
                            pe[:], wae[:, k, m * 128:(m + 1) * 128],
                            encT[:, k, ns], start=(k == 0), stop=(k == 3))
                    if n2 == 0:
                        nc.scalar.activation(encP[:, m, ns], pe[:], AF.Copy)
                    else:
                        nc.vector.tensor_copy(encP[:, m, ns], pe[:])


            def gru_math(prz, pn, phn, h_old, lname):
                """transposed-layout GRU cell update; returns new hT (bf16)."""
                t_rz = wk.tile([128, 8, BS], bf16, tag=f"trz{lname}")
                nc.scalar.activation(t_rz[:], prz[:], AF.Tanh, scale=1.0 / 128)
                tmp1 = wk.tile([128, 4, BS], f32, tag=f"tm1{lname}")
                nc.vector.scalar_tensor_tensor(
                    tmp1[:], t_rz[:, 0:4, :], 1.0, phn[:], OP.add, OP.mult)
                a2 = wk.tile([128, 4, BS], f32, tag=f"a2{lname}")
                nc.vector.tensor_add(a2[:], tmp1[:], pn[:])
                n_t = wk.tile([128, 4, BS], bf16, tag=f"n{lname}")
                nc.scalar.activation(n_t[:], a2[:], AF.Tanh, scale=1.0 / 64)
                v_t = wk.tile([128, 4, BS], f32, tag=f"v{lname}")
                nc.vector.tensor_sub(v_t[:], h_old[:], n_t[:])
                tmp2 = wk.tile([128, 4, BS], f32, tag=f"tm2{lname}")
                nc.vector.scalar_tensor_tensor(
                    tmp2[:], t_rz[:, 4:8, :], 1.0, v_t[:], OP.add, OP.mult)
                h_new = st.tile([128, 4, BS], bf16, tag=f"h{lname}T")
                nc.vector.scalar_tensor_tensor(
                    h_new[:], tmp2[:], 0.5, n_t[:], OP.mult, OP.add)
                return h_new

            # ================= time loop =================
            for t in range(DEC):
                patt, q_sb = patt_c, q_sb_c

                # --- en = tanh((encP64 + q64)/64), chunked; m3 add on
                # Pool, m0-2 on DVE; tanh order = readiness order ---
                ea = wk.tile([128, 4, BS * E], bf16, tag="ea")
                en = wk.tile([128, 4, BS * E], bf16, tag="en")
                pso = pss.tile([128, 48], f32, tag="pso")
                scT = pso[0:E, 0:BS]
                def emit_add(m):
                    if m == 0:
                        # halves: the first tanh starts half an add earlier
                        for cs in (slice(0, 4 * E), slice(4 * E, 8 * E)):
                            nc.vector.tensor_add(
                                ea[:, 0, cs].rearrange("p (e b) -> p e b",
                                                       b=BS),
                                encP[:, 0, cs].rearrange("p (e b) -> p e b",
                                                         b=BS),
                                q_sb[:, 0, :].unsqueeze(1).broadcast_to(
                                    (128, E // 2, BS)))
                        return
                    nc.vector.tensor_add(
                        ea[:, m, :].rearrange("p (e b) -> p e b", e=E),
                        encP[:, m, :].rearrange("p (e b) -> p e b", e=E),
                        q_sb[:, m, :].unsqueeze(1).broadcast_to((128, E, BS)))

                morder = (0, 1, 2, 3)
                if t == 0:
                    # interleave the one-time encP build with t0's chunks
                    for m in morder:
                        emit_encP(m)
                        emit_add(m)
                else:
                    for m in (0, 1, 2):
                        emit_add(m)
                    emit_add(3)
                nc.scalar.activation(en[:, 0, 0:4 * E], ea[:, 0, 0:4 * E],
                                     AF.Tanh, scale=1.0 / 64)
                nc.scalar.activation(en[:, 0, 4 * E:], ea[:, 0, 4 * E:],
                                     AF.Tanh, scale=1.0 / 64)
                for m in (1, 2, 3):
                    nc.scalar.activation(en[:, m, :], ea[:, m, :],
                                         AF.Tanh, scale=1.0 / 64)
                enb = en[:].rearrange("p m (e b) -> p m b e", b=BS)
                for b in range(BS):
                    for ki in range(4):
                        nc.tensor.matmul(
                            scT[:, b:b + 1], enb[:, ki, b, :],
                            v_sb[:, ki, 0, 0:1],
                            start=(ki == 0), stop=(ki == 3))

                # --- softmax over E, transposed: exp [96, 8]; z via
                # all-ones matmul onto all 128 partitions; 1/z folded into
                # the wsT copy ---
                expw = wk.tile([E, BS], bf16, tag="expw")
                nc.scalar.activation(expw[:], scT[:], AF.Exp)
                pzs = pso[:, 32:40]
                nc.tensor.matmul(pzs[:], ones96[:], expw[:],
                                 start=True, stop=True)
                zrP = wk.tile([128, BS], f32, tag="zrP")
                nc.vector.reciprocal(zrP[:], pzs[:])

                # --- wsT[:, m, b] = (enc[b].T-chunk @ expw_b) / z_b ---
                pws = patt[:, 4:8, :]
                for b in range(BS):
                    for m in range(4):
                        nc.tensor.matmul(pws[:, m, b:b + 1],
                                         encE[:, b, m, :],
                                         expw[:, b:b + 1],
                                         start=True, stop=True)
                wsT = wk.tile([128, 4, BS], bf16, tag="wsT")
                nc.vector.tensor_mul(
                    wsT[:], pws[:],
                    zrP[:].unsqueeze(1).broadcast_to((128, 4, BS)))

                # --- GRU layer 0 ---
                rhs0 = [h0T[:, k, :] for k in range(4)] + \
                       [wsT[:, k, :] for k in range(4)] + [cb[:]]
                pg0 = pss.tile([128, 16, BS], f32, tag="g0")
                prz0 = pg0[:, 0:8, :]
                pn0 = pg0[:, 8:12, :]
                phn0 = pg0[:, 12:16, :]
                # h/cb chunks first so PE starts before wsT lands
                rz_ord = [0, 1, 2, 3, 8, 4, 5, 6, 7]
                for m in range(4):
                    for j, i in enumerate(rz_ord):
                        nc.tensor.matmul(
                            prz0[:, m, :], al0[:, i, m * 128:(m + 1) * 128],
                            rhs0[i], start=(j == 0), stop=(j == 8))
                    for j, i in enumerate(rz_ord):
                        nc.tensor.matmul(
                            prz0[:, 4 + m, :],
                            al0[:, 9 + i, m * 128:(m + 1) * 128],
                            rhs0[i], start=(j == 0), stop=(j == 8))
                    for j, (ci, rh) in enumerate(
                            [(4, rhs0[8]), (0, rhs0[4]), (1, rhs0[5]),
                             (2, rhs0[6]), (3, rhs0[7])]):
                        nc.tensor.matmul(
                            pn0[:, m, :], al0[:, 18 + ci, m * 128:(m + 1) * 128],
                            rh, start=(j == 0), stop=(j == 4))
                    for i, rh in enumerate([rhs0[0], rhs0[1], rhs0[2],
                                            rhs0[3], rhs0[8]]):
                        nc.tensor.matmul(
                            phn0[:, m, :], al0[:, 23 + i, m * 128:(m + 1) * 128],
                            rh, start=(i == 0), stop=(i == 4))
                h0T = gru_math(prz0, pn0, phn0, h0T, "0")

                # --- GRU layer 1 ---
                rhs1 = [h0T[:, k, :] for k in range(4)] + \
                       [h1T[:, k, :] for k in range(4)] + [cb[:]]
                pg1 = pss.tile([128, 16, BS], f32, tag="g1")
                prz1 = pg1[:, 0:8, :]
                pn1 = pg1[:, 8:12, :]
                phn1 = pg1[:, 12:16, :]
                for m in range(4):
                    for i in range(9):
                        nc.tensor.matmul(
                            prz1[:, m, :], al1[:, i, m * 128:(m + 1) * 128],
                            rhs1[i], start=(i == 0), stop=(i == 8))
                    for i in range(9):
                        nc.tensor.matmul(
                            prz1[:, 4 + m, :],
                            al1[:, 9 + i, m * 128:(m + 1) * 128],
                            rhs1[i], start=(i == 0), stop=(i == 8))
                    for i, rh in enumerate([rhs1[0], rhs1[1], rhs1[2],
                                            rhs1[3], rhs1[8]]):
                        nc.tensor.matmul(
                            pn1[:, m, :], al1[:, 18 + i, m * 128:(m + 1) * 128],
                            rh, start=(i == 0), stop=(i == 4))
                    for i, rh in enumerate([rhs1[4], rhs1[5], rhs1[6],
                                            rhs1[7], rhs1[8]]):
                        nc.tensor.matmul(
                            phn1[:, m, :], al1[:, 23 + i, m * 128:(m + 1) * 128],
                            rh, start=(i == 0), stop=(i == 4))
                h1T = gru_math(prz1, pn1, phn1, h1T, "1")

                # --- q for t+1 first (only needs h1T), then out proj ---
                if t < DEC - 1:
                    patt_c, q_sb_c = emit_q(h1T)
                po = pso[0:T, 40:48]  # [4, 8] region
                rhso = [h1T[:, k, :] for k in range(4)] + \
                       [wsT[:, k, :] for k in range(4)] + [cb[:]]
                for i in range(9):
                    nc.tensor.matmul(po[:], wo[:, i, :], rhso[i],
                                     start=(i == 0), stop=(i == 8))
                nc.vector.tensor_copy(outT[:, t, :], po[:])

                # --- cur update for next step ---
                if t < DEC - 1:
                    nc.gpsimd.tensor_copy(cb[0:32, :], inT[:, t, :])
                    nc.vector.tensor_copy(cb[0:T, :], po[:])

            nc.sync.dma_start(out_e[:], outT[:])

    # --- post-pass: cap every instruction at one sync wait by hoisting
    # extras onto same-engine NoOps inserted just before it. ---
    from concourse import mybir
    ctr = 0
    f = nc.m.functions[0]
    for blk in f.blocks:
        il = blk.instructions
        i = 0
        while i < len(il):
            inst = il[i]
            si = inst.sync_info
            waits = list(si.on_wait) if si is not None and si.on_wait else []
            if len(waits) > 1:
                SyncInfo = type(si)
                inst.sync_info = SyncInfo(
                    on_wait=[waits[-1]], on_update=list(si.on_update or []))
                for w in waits[:-1]:
                    nop = mybir.InstNoOp(name=f"I-nopw-{ctr}")
                    ctr += 1
                    nop.engine = inst.engine
                    nop.sync_info = SyncInfo(on_wait=[w], on_update=[])
                    nc.register_instruction(nop)
                    il.insert(i, nop)
                    i += 1
            i += 1

    return nc


def _prep_inputs(inputs, hidden, enc_outputs, target_indices,
                 W_attn, b_attn, v_attn,
                 gru_Wi0, gru_Wh0, gru_bi0, gru_bh0,
                 gru_Wi1, gru_Wh1, gru_bi1, gru_bh1,
                 W_out, b_out):
    """Build per-core input maps (host-side layout prep only)."""
    ti = np.asarray(target_indices)
    assert np.array_equal(ti, np.arange(T)), \
        "kernel specialized for target_indices == arange(T)"

    W_attn = np.asarray(W_attn, np.float32)
    Wa_h, Wa_e = W_attn[:, :H], W_attn[:, H:]
    Wi0 = np.asarray(gru_Wi0, np.float32); Wh0 = np.asarray(gru_Wh0, np.float32)
    bi0 = np.asarray(gru_bi0, np.float32); bh0 = np.asarray(gru_bh0, np.float32)
    Wi1 = np.asarray(gru_Wi1, np.float32); Wh1 = np.asarray(gru_Wh1, np.float32)
    bi1 = np.asarray(gru_bi1, np.float32); bh1 = np.asarray(gru_bh1, np.float32)
    W_out = np.asarray(W_out, np.float32); b_out = np.asarray(b_out, np.float32)

    # attention weights: aw[p, k, m*128+c]: k 0-3 = Wa_h, 4-7 = Wa_e
    wah = np.zeros((128, 4, H), np.float32)
    wae = np.zeros((128, 4, H), np.float32)
    for k in range(4):
        wah[:, k, :] = Wa_h[:, k * 128:(k + 1) * 128].T
        wae[:, k, :] = Wa_e[:, k * 128:(k + 1) * 128].T
    wah = _f8(wah * 64.0)
    wae = _f8(wae * 64.0)

    # block-diag v
    v_rk = np.asarray(v_attn, np.float32).reshape(4, 128).T   # [128, 4]
    v_h = np.zeros((128, 4, 4, BS), np.float32)
    for p in range(4):
        v_h[:, :, p, 2 * p] = v_rk
        v_h[:, :, p, 2 * p + 1] = v_rk
    v_h = _bf16(v_h)

    def gate_chunks(Wh_part, Wi_ws, Wi_cur, bias_row, scale):
        """9 chunks: h(4) + ws(4) + cb; or pass None to skip groups."""
        out = []
        if Wh_part is not None:
            for k in range(4):
                out.append(scale * Wh_part[:, k * 128:(k + 1) * 128].T)
        if Wi_ws is not None:
            for k in range(4):
                out.append(scale * Wi_ws[:, k * 128:(k + 1) * 128].T)
        cbch = np.zeros((128, Wh_part.shape[0] if Wh_part is not None
                         else Wi_ws.shape[0]), np.float32)
        if Wi_cur is not None:
            cbch[0:F, :] = scale * Wi_cur.T
        cbch[F, :] = scale * bias_row
        out.append(cbch)
        return out

    # layer 0: contraction z-order [h0(4) | ws(4) | cb]
    l0 = []
    l0 += gate_chunks(Wh0[0:512], Wi0[0:512, F:], Wi0[0:512, :F],
                      bi0[0:512] + bh0[0:512], 64.0)            # r: 9
    l0 += gate_chunks(Wh0[512:1024], Wi0[512:1024, F:], Wi0[512:1024, :F],
                      bi0[512:1024] + bh0[512:1024], 64.0)      # z: 9
    l0 += gate_chunks(None, Wi0[1024:1536, F:], Wi0[1024:1536, :F],
                      bi0[1024:1536], 64.0)                     # ni: 5
    l0 += gate_chunks(Wh0[1024:1536], None, None,
                      bh0[1024:1536], 32.0)                     # nh: 5
    al0 = _f8(np.stack(l0, axis=1))                             # [128, 28, 512]

    # layer 1: z-order [h0'(4) | h1(4) | cb(bias only)]
    l1 = []
    l1 += gate_chunks(Wi1[0:512], Wh1[0:512], None,
                      bi1[0:512] + bh1[0:512], 64.0)            # r: 9
    l1 += gate_chunks(Wi1[512:1024], Wh1[512:1024], None,
                      bi1[512:1024] + bh1[512:1024], 64.0)      # z: 9
    l1 += gate_chunks(Wi1[1024:1536], None, None, bi1[1024:1536], 64.0)  # ni: 5
    l1 += gate_chunks(Wh1[1024:1536], None, None, bh1[1024:1536], 32.0)  # nh: 5
    al1 = _f8(np.stack(l1, axis=1))

    # out projection chunks [h1(4) | ws(4) | cb]
    woc = []
    for k in range(4):
        woc.append(W_out[:, k * 128:(k + 1) * 128].T)
    for k in range(4):
        woc.append(W_out[:, H + k * 128:H + (k + 1) * 128].T)
    cbo = np.zeros((128, T), np.float32)
    cbo[0:F, :] = W_out[:, 2 * H:2 * H + F].T
    cbo[F, :] = b_out
    woc.append(cbo)
    wo = _bf16(np.stack(woc, axis=1))                           # [128, 9, 4]

    battn = _bf16(b_attn[None, :] * 64.0)
    ones8 = _bf16(np.ones((1, BS), np.float32))
    ones96 = _bf16(np.ones((E, 128), np.float32))

    inputs = np.asarray(inputs, np.float32)
    hidden = np.asarray(hidden, np.float32)
    enc_outputs = np.asarray(enc_outputs, np.float32)

    in_maps = []
    for c in range(N_CORES):
        s = slice(c * BS, (c + 1) * BS)
        encc = enc_outputs[s]                      # [8, 96, 512]
        # encT columns (e, b)-major so the q-broadcast add has b packed
        encT = _bf16(encc.transpose(2, 1, 0).reshape(4, 128, E * BS)
                     .transpose(1, 0, 2))
        # encE[e, b, m, c] = enc[b, e, m*128+c]
        encE = _bf16(encc.reshape(BS, E, 4, 128).transpose(1, 0, 2, 3))
        h0 = hidden[0, s]                          # [8, 512]
        h1 = hidden[1, s]
        cb0 = np.zeros((128, BS), np.float32)
        cb0[0:F, :] = inputs[s, 0, :].T
        cb0[F, :] = 1.0
        in_maps.append({
            "encT": encT, "encE": encE,
            "wae": wae, "wah": wah,
            "battn": battn, "ones8": ones8, "ones96": ones96,
            "v": v_h, "al0": al0, "al1": al1, "wo": wo,
            "h0T": _bf16(h0.T.reshape(4, 128, BS).transpose(1, 0, 2)),
            "h1T": _bf16(h1.T.reshape(4, 128, BS).transpose(1, 0, 2)),
            "cb0": _bf16(cb0),
            "inT": _bf16(inputs[s].transpose(2, 1, 0)),
        })
    return in_maps


def get_nc():
    if "nc" not in _COMPILED:
        _COMPILED["nc"] = build_nc()
    return _COMPILED["nc"]


def kernel(**inputs):
    from concourse.bass_utils import run_bass_kernel_spmd
    nc = get_nc()
    in_maps = _prep_inputs(**inputs)
    res = run_bass_kernel_spmd(nc, in_maps, list(range(N_CORES)))
    # out_e is [T, DEC, BS] per core -> full [B, DEC, T]
    out = np.concatenate(
        [res.results[c]["out"].transpose(2, 1, 0) for c in range(N_CORES)],
        axis=0)
    return np.ascontiguousarray(out, dtype=np.float32)
